# revision 3
# baseline (speedup 1.0000x reference)
"""Cross-attention kernel for 8 TRN2 NeuronCores (Bass/Tile, SPMD).

Problem: B=4, SQ=SKV=2048, D_MODEL=1024, H=16 heads, Dh=64, fp32 I/O.
    Q = q @ Wq.T + bq; K = kv @ Wk.T + bk; V = kv @ Wv.T + bv
    out = softmax(Q K^T / sqrt(Dh)) V  -> concat heads -> @ Wo.T + bo

Sharding: 8 cores = 4 batches x 2 head-groups (8 heads each). Each core
computes its batch's projections for its 8 heads, full attention for those
heads, and a partial out-projection (its 512 columns of the head-concat dim).
The host sums the two partials per batch (no device collectives needed).

v3 structure (vs v2): the exp activations bound the attention phase
(~1038ns per 2-head step vs ~854ns of PE work), so the Q- and
out-projections are streamed into the attention phase to fill the PE slack:

  phase 1: K/V projections (+ Q for the first q-chunk)
  phase 2: for each q-chunk jc: emit out-proj(jc-1), Q-proj(jc+1), then
           attention for all 4 head-pairs at jc
  tail:    out-proj for the last q-chunk

To free the 2 PSUM banks the interleaved projections need, PV accumulators
are single-buffered and copied (with their denominator row) to SBUF by the
DVE right after the last accumulation; normalization runs later from the
SBUF copy, off the PV critical path.

Device layout:
  - inputs qT/kvT: (1024, 2048) = x[b].T in bf16
  - QT, KT: (512, 2048) bf16, heads-major (8*64 rows)
  - Vhat: (2048, 520) bf16 = per head [V_h (64 cols) | 1.0]; the ones column
    makes the PV matmul emit the softmax denominator as row 64
  - scores: row-tiled concurrent matmul pairs (two heads in PE row groups
    0-63/64-127, contract dim Dh=64) -> one [128, 1024] PSUM set so exp
    runs at FD=1024; P written in bf16 straight from the activation
  - normalize: reciprocal of row 64 of the SBUF PV copy, partition-broadcast
    via DRAM bounce, multiply rows 0..63 -> attnT (512, 2048) bf16
  - out-projection: attnT chunks stationary x woT moving -> out (2048, 1024)
    fp32 partial, bias added on head-group-0 cores only.
"""

import numpy as np
import ml_dtypes

B = 4
S = 2048          # SQ == SKV
D = 1024
H_PER_CORE = 8
DH = 64
DC = H_PER_CORE * DH            # 512 head-concat dims per core
DHP = DH + 1                    # V-hat column block per head (64 + ones col)
N_CORES = 8

_CACHE = {}


def _build_program(repeat=1):
    import concourse.bass as bass
    import concourse.tile as tile
    from concourse import bacc, mybir

    f32 = mybir.dt.float32
    bf16 = mybir.dt.bfloat16
    nc = bacc.Bacc("TRN2", target_bir_lowering=False, debug=False,
                   enable_asserts=False, num_devices=N_CORES)

    qT = nc.dram_tensor("qT", [D, S], bf16, kind="ExternalInput").ap()
    kvT = nc.dram_tensor("kvT", [D, S], bf16, kind="ExternalInput").ap()
    wqT = nc.dram_tensor("wqT", [D, DC], bf16, kind="ExternalInput").ap()
    wkT = nc.dram_tensor("wkT", [D, DC], bf16, kind="ExternalInput").ap()
    wvh = nc.dram_tensor("wvh", [D, H_PER_CORE * DHP], bf16, kind="ExternalInput").ap()
    bq = nc.dram_tensor("bq", [DC], f32, kind="ExternalInput").ap()
    bk = nc.dram_tensor("bk", [DC], f32, kind="ExternalInput").ap()
    bvh = nc.dram_tensor("bvh", [H_PER_CORE * DHP], f32, kind="ExternalInput").ap()
    woT = nc.dram_tensor("woT", [DC, D], bf16, kind="ExternalInput").ap()
    bo = nc.dram_tensor("bo", [D], f32, kind="ExternalInput").ap()
    out = nc.dram_tensor("out", [S, D], f32, kind="ExternalOutput").ap()

    VW = H_PER_CORE * DHP       # 520
    KC = D // 128               # 8 contraction chunks for projections
    NM = DC // 128              # 4 partition chunks of QT/KT (head pairs)
    JW = 512                    # q-chunk width
    NJ = S // JW                # 4 q-chunks
    NSB = S // 128              # 16 s-blocks

    with tile.TileContext(nc) as tc:
      def _emit():
        with tc.tile_pool(name="persist", bufs=1) as persist, \
             tc.tile_pool(name="wqo", bufs=1) as wqo, \
             tc.tile_pool(name="xq", bufs=2) as xq, \
             tc.tile_pool(name="ppq", bufs=2, space="PSUM") as ppq, \
             tc.tile_pool(name="otp", bufs=3) as otp:
            qt_t = [persist.tile([128, S], bf16, tag=f"qt{m}", name=f"qt{m}") for m in range(NM)]
            kt_t = [persist.tile([128, S], bf16, tag=f"kt{m}", name=f"kt{m}") for m in range(NM)]
            vh_t = [persist.tile([128, VW], bf16, tag=f"vh{sb}", name=f"vh{sb}") for sb in range(NSB)]
            at_t = [persist.tile([128, S], bf16, tag=f"at{m}", name=f"at{m}") for m in range(NM)]

            # biases: bq/bk as (128, NM) per-partition scalars; bvh broadcast
            bq_t = persist.tile([128, NM], f32, tag="bq")
            bk_t = persist.tile([128, NM], f32, tag="bk")
            bvh_t = persist.tile([128, VW], f32, tag="bvh")
            bo_t = persist.tile([128, D], f32, tag="bo")

            def col_ap(vec, n):  # (n*128,) dram vector -> (128, n) column tile ap
                return bass.AP(tensor=vec.tensor, offset=vec.offset,
                               ap=[[1, 128], [128, n]])

            def bcast_ap(vec, p, w):  # (w,) dram vector -> (p, w) broadcast
                return bass.AP(tensor=vec.tensor, offset=vec.offset,
                               ap=[[0, p], [1, w]])

            nc.sync.dma_start(out=bq_t, in_=col_ap(bq, NM))
            nc.sync.dma_start(out=bk_t, in_=col_ap(bk, NM))
            nc.sync.dma_start(out=bvh_t, in_=bcast_ap(bvh, 128, VW))
            nc.sync.dma_start(out=bo_t, in_=bcast_ap(bo, 128, D))

            # weights that live through the whole kernel: Wq + Wo
            wq_t = [wqo.tile([128, DC], bf16, tag=f"wq{k}", name=f"wq{k}") for k in range(KC)]
            wo_t = [wqo.tile([128, D], bf16, tag=f"wo{k}", name=f"wo{k}") for k in range(NM)]
            for k in range(KC):
                nc.sync.dma_start(out=wq_t[k], in_=wqT[k * 128:(k + 1) * 128, :])
            for k in range(NM):
                nc.sync.dma_start(out=wo_t[k], in_=woT[k * 128:(k + 1) * 128, :])

            def q_proj(jc):
                """Project Q columns for q-chunk jc into qt_t[:, jsl]."""
                jsl = slice(jc * JW, (jc + 1) * JW)
                q_c = [xq.tile([128, JW], bf16, tag=f"q{k}", name=f"q{k}")
                       for k in range(KC)]
                for k in range(KC):
                    nc.sync.dma_start(out=q_c[k], in_=qT[k * 128:(k + 1) * 128, jsl])
                for m in range(NM):
                    msl = slice(m * 128, (m + 1) * 128)
                    ps = ppq.tile([128, JW], f32, tag="proj")
                    for k in range(KC):
                        nc.tensor.matmul(ps, wq_t[k][:, msl], q_c[k],
                                         start=(k == 0), stop=(k == KC - 1))
                    nc.vector.tensor_scalar_add(qt_t[m][:, jsl], ps, bq_t[:, m:m + 1])

            def out_proj(jc):
                """Partial out-projection for the q-rows of chunk jc."""
                for qm in range(jc * (JW // 128), (jc + 1) * (JW // 128)):
                    qsl = slice(qm * 128, (qm + 1) * 128)
                    for n in range(D // 512):
                        nsl = slice(n * 512, (n + 1) * 512)
                        po = ppq.tile([128, 512], f32, tag="proj")
                        for k in range(NM):
                            nc.tensor.matmul(po, at_t[k][:, qsl], wo_t[k][:, nsl],
                                             start=(k == 0), stop=(k == NM - 1))
                        o_t = otp.tile([128, 512], f32, tag="o")
                        nc.vector.tensor_add(o_t, po, bo_t[:, nsl])
                        nc.sync.dma_start(out=out[qsl, nsl], in_=o_t)

            # ---- phase 1: K/V projections over 4 s-quarters, then Q(jc=0) --
            SQW = 512
            with tc.tile_pool(name="wkv", bufs=1) as wkv, \
                 tc.tile_pool(name="xkv", bufs=1) as xkv, \
                 tc.tile_pool(name="ppv", bufs=2, space="PSUM") as ppv:
                wk_t = [wkv.tile([128, DC], bf16, tag=f"wk{k}", name=f"wk{k}") for k in range(KC)]
                wv_t = [wkv.tile([128, VW], bf16, tag=f"wv{k}", name=f"wv{k}") for k in range(KC)]
                for k in range(KC):
                    nc.sync.dma_start(out=wk_t[k], in_=wkT[k * 128:(k + 1) * 128, :])
                    nc.sync.dma_start(out=wv_t[k], in_=wvh[k * 128:(k + 1) * 128, :])

                for sq in range(S // SQW):
                    ssl = slice(sq * SQW, (sq + 1) * SQW)
                    kv_c = [xkv.tile([128, SQW], bf16, tag=f"kv{k}", name=f"kv{k}")
                            for k in range(KC)]
                    for k in range(KC):
                        nc.sync.dma_start(out=kv_c[k], in_=kvT[k * 128:(k + 1) * 128, ssl])

                    for m in range(NM):
                        msl = slice(m * 128, (m + 1) * 128)
                        ps = ppq.tile([128, SQW], f32, tag="proj")
                        for k in range(KC):
                            nc.tensor.matmul(ps, wk_t[k][:, msl], kv_c[k],
                                             start=(k == 0), stop=(k == KC - 1))
                        nc.vector.tensor_scalar_add(kt_t[m][:, ssl], ps, bk_t[:, m:m + 1])
                    for sm in range(SQW // 128):
                        sb = sq * (SQW // 128) + sm
                        smsl = slice(sm * 128, (sm + 1) * 128)
                        psv = ppv.tile([128, 1024], f32, tag="vproj")
                        for k in range(KC):
                            nc.tensor.matmul(psv[:, 0:512], kv_c[k][:, smsl], wv_t[k][:, 0:512],
                                             start=(k == 0), stop=(k == KC - 1))
                            nc.tensor.matmul(psv[:, 512:VW], kv_c[k][:, smsl], wv_t[k][:, 512:VW],
                                             start=(k == 0), stop=(k == KC - 1))
                        nc.vector.tensor_add(vh_t[sb], psv[:, 0:VW], bvh_t)

            q_proj(0)

            # ---- phase 2: attention with interleaved Q/out projections ----
            with tc.tile_pool(name="sps", bufs=2, space="PSUM") as sps, \
                 tc.tile_pool(name="pvs", bufs=1, space="PSUM") as pvs, \
                 tc.tile_pool(name="pt", bufs=3) as ptp, \
                 tc.tile_pool(name="pvc", bufs=2) as pvcp, \
                 tc.tile_pool(name="nrm", bufs=3) as nrm, \
                 tc.tile_pool(name="dscr", bufs=3, space="DRAM") as dscr:
                for jc in range(NJ):
                    jsl = slice(jc * JW, (jc + 1) * JW)
                    if jc > 0:
                        out_proj(jc - 1)
                    if jc + 1 < NJ:
                        q_proj(jc + 1)
                    for hp in range(NM):
                        v0 = slice(2 * hp * DHP, 2 * hp * DHP + DHP)
                        v1 = slice((2 * hp + 1) * DHP, (2 * hp + 1) * DHP + DHP)
                        pv0 = pvs.tile([DHP, JW], f32, tag="pv0")
                        pv1 = pvs.tile([DHP, JW], f32, tag="pv1")
                        for sb in range(NSB):
                            sbsl = slice(sb * 128, (sb + 1) * 128)
                            sp = sps.tile([128, 2 * JW], f32, tag="sc")
                            # two heads concurrently in PE row groups 0/64
                            nc.tensor.matmul(sp[:, 0:JW],
                                             kt_t[hp][0:64, sbsl],
                                             qt_t[hp][0:64, jsl],
                                             start=True, stop=True)
                            nc.tensor.matmul(sp[:, JW:2 * JW],
                                             kt_t[hp][64:128, sbsl],
                                             qt_t[hp][64:128, jsl],
                                             start=True, stop=True)
                            p_t = ptp.tile([128, 2 * JW], bf16, tag="p")
                            nc.scalar.activation(p_t, sp, mybir.ActivationFunctionType.Exp,
                                                 scale=0.125)
                            nc.tensor.matmul(pv0, vh_t[sb][:, v0], p_t[:, 0:JW],
                                             start=(sb == 0), stop=(sb == NSB - 1))
                            nc.tensor.matmul(pv1, vh_t[sb][:, v1], p_t[:, JW:2 * JW],
                                             start=(sb == 0), stop=(sb == NSB - 1))
                        # free the PSUM accumulators fast: raw copy to SBUF,
                        # normalize later from the copy
                        pvc0 = pvcp.tile([DHP, JW], f32, tag="pvc0")
                        pvc1 = pvcp.tile([DHP, JW], f32, tag="pvc1")
                        nc.vector.tensor_copy(pvc0, pv0)
                        nc.vector.tensor_copy(pvc1, pv1)
                        for h, pvc in ((0, pvc0), (1, pvc1)):
                            hsl = slice(h * 64, h * 64 + 64)
                            rec = nrm.tile([1, JW], f32, tag=f"rec{h}")
                            nc.vector.reciprocal(rec, pvc[64:65, :])
                            scr = dscr.tile([1, JW], f32, tag=f"scr{h}")
                            nc.sync.dma_start(out=scr, in_=rec)
                            recb = nrm.tile([64, JW], f32, tag=f"recb{h}")
                            sc = scr[0, :]
                            nc.sync.dma_start(
                                out=recb,
                                in_=bass.AP(tensor=sc.tensor, offset=sc.offset,
                                            ap=[[0, 64]] + sc.ap))
                            nc.vector.tensor_mul(at_t[hp][hsl, jsl], pvc[0:64, :], recb)
                out_proj(NJ - 1)

      if repeat > 1:
          with tc.For_i(0, repeat, 1):
              _emit()
      else:
          _emit()

    nc.compile()
    return nc


def _get_runner(repeat=1):
    """Build the program once and return a cached jitted SPMD runner."""
    key = ("runner", repeat)
    if key in _CACHE:
        return _CACHE[key]

    import jax
    import jax.numpy as jnp
    from jax.sharding import Mesh, PartitionSpec
    from jax.experimental.shard_map import shard_map
    from concourse import mybir
    from concourse.bass2jax import (_bass_exec_p, install_neuronx_cc_hook,
                                    partition_id_tensor)

    nc = _build_program(repeat)
    install_neuronx_cc_hook()

    partition_name = nc.partition_id_tensor.name if nc.partition_id_tensor else None
    in_names, out_names, out_avals, zero_shapes = [], [], [], []
    for alloc in nc.m.functions[0].allocations:
        if not isinstance(alloc, mybir.MemoryLocationSet):
            continue
        name = alloc.memorylocations[0].name
        if alloc.kind == "ExternalInput":
            if name != partition_name:
                in_names.append(name)
        elif alloc.kind == "ExternalOutput":
            out_names.append(name)
            shape = tuple(alloc.tensor_shape)
            dtype = mybir.dt.np(alloc.dtype)
            out_avals.append(jax.core.ShapedArray(shape, dtype))
            zero_shapes.append((shape, dtype))
    n_params = len(in_names)
    n_outs = len(out_avals)
    all_in_names = list(in_names) + list(out_names)
    if partition_name is not None:
        all_in_names.append(partition_name)
    donate = tuple(range(n_params, n_params + n_outs))

    def _body(*args):
        operands = list(args)
        if partition_name is not None:
            operands.append(partition_id_tensor())
        outs = _bass_exec_p.bind(
            *operands,
            out_avals=tuple(out_avals),
            in_names=tuple(all_in_names),
            out_names=tuple(out_names),
            lowering_input_output_aliases=(),
            sim_require_finite=True,
            sim_require_nnan=True,
            nc=nc,
        )
        return tuple(outs)

    devices = jax.devices()[:N_CORES]
    mesh = Mesh(np.asarray(devices), ("core",))
    in_specs = (PartitionSpec("core"),) * (n_params + n_outs)
    out_specs = (PartitionSpec("core"),) * n_outs
    sharded = jax.jit(
        shard_map(_body, mesh=mesh, in_specs=in_specs, out_specs=out_specs,
                  check_rep=False),
        donate_argnums=donate, keep_unused=True)

    def run(in_maps):
        concat_in = [np.concatenate([np.asarray(m[name]) for m in in_maps], axis=0)
                     for name in in_names]
        concat_zeros = [np.zeros((N_CORES * s[0], *s[1:]), d) for s, d in zero_shapes]
        out_arrs = sharded(*concat_in, *concat_zeros)
        out_arrs = [np.asarray(a) for a in jax.block_until_ready(out_arrs)]
        return [
            {name: out_arrs[i].reshape(N_CORES, *out_avals[i].shape)[c]
             for i, name in enumerate(out_names)}
            for c in range(N_CORES)
        ]

    _CACHE[("internals", repeat)] = {
        "sharded": sharded, "mesh": mesh, "in_names": in_names,
        "out_names": out_names, "zero_shapes": zero_shapes, "nc": nc,
    }
    _CACHE[key] = run
    return run


def _prep_in_maps(query, key_value, Wq, bq, Wk, bk, Wv, bv, Wo, bo):
    f = np.float32
    bf = ml_dtypes.bfloat16
    in_maps = []
    for c in range(N_CORES):
        b, hg = c // 2, c % 2
        sl = slice(hg * DC, (hg + 1) * DC)
        wv_s = np.asarray(Wv, f)[sl, :].T.reshape(D, H_PER_CORE, DH)
        wvh = np.concatenate([wv_s, np.zeros((D, H_PER_CORE, 1), f)], axis=2)
        bv_s = np.asarray(bv, f)[sl].reshape(H_PER_CORE, DH)
        bvh = np.concatenate([bv_s, np.ones((H_PER_CORE, 1), f)], axis=1)
        in_maps.append({
            "qT": np.ascontiguousarray(np.asarray(query, f)[b].T).astype(bf),
            "kvT": np.ascontiguousarray(np.asarray(key_value, f)[b].T).astype(bf),
            "wqT": np.ascontiguousarray(np.asarray(Wq, f)[sl, :].T).astype(bf),
            "wkT": np.ascontiguousarray(np.asarray(Wk, f)[sl, :].T).astype(bf),
            "wvh": np.ascontiguousarray(wvh.reshape(D, H_PER_CORE * DHP)).astype(bf),
            "bq": np.ascontiguousarray(np.asarray(bq, f)[sl]),
            "bk": np.ascontiguousarray(np.asarray(bk, f)[sl]),
            "bvh": np.ascontiguousarray(bvh.reshape(H_PER_CORE * DHP)),
            "woT": np.ascontiguousarray(np.asarray(Wo, f)[:, sl].T).astype(bf),
            "bo": (np.asarray(bo, f) if hg == 0 else np.zeros(D, f)),
        })
    return in_maps


def kernel(query, key_value, Wq, bq, Wk, bk, Wv, bv, Wo, bo):
    run = _get_runner()
    in_maps = _prep_in_maps(query, key_value, Wq, bq, Wk, bk, Wv, bv, Wo, bo)
    results = run(in_maps)
    out = np.empty((B, S, D), np.float32)
    for b in range(B):
        out[b] = results[2 * b]["out"] + results[2 * b + 1]["out"]
    return out


# revision 7
# speedup vs baseline: 1.1971x; 1.1971x over previous
"""Cross-attention kernel for 8 TRN2 NeuronCores (Bass/Tile, SPMD).

Problem: B=4, SQ=SKV=2048, D_MODEL=1024, H=16 heads, Dh=64, fp32 I/O.
    Q = q @ Wq.T + bq; K = kv @ Wk.T + bk; V = kv @ Wv.T + bv
    out = softmax(Q K^T / sqrt(Dh)) V  -> concat heads -> @ Wo.T + bo

Sharding: 8 cores = 4 batches x 2 head-groups (8 heads each). Each core
computes its batch's projections for its 8 heads, full attention for those
heads, and a partial out-projection (its 512 columns of the head-concat dim).
The host sums the two partials per batch (no device collectives needed).

v3 structure (vs v2): the exp activations bound the attention phase
(~1038ns per 2-head step vs ~854ns of PE work), so the Q- and
out-projections are streamed into the attention phase to fill the PE slack:

  phase 1: K/V projections (+ Q for the first q-chunk)
  phase 2: for each q-chunk jc: emit out-proj(jc-1), Q-proj(jc+1), then
           attention for all 4 head-pairs at jc
  tail:    out-proj for the last q-chunk

To free the 2 PSUM banks the interleaved projections need, PV accumulators
are single-buffered and copied (with their denominator row) to SBUF by the
DVE right after the last accumulation; normalization runs later from the
SBUF copy, off the PV critical path.

Device layout:
  - inputs qT/kvT: (1024, 2048) = x[b].T in bf16
  - QT, KT: (512, 2048) bf16, heads-major (8*64 rows)
  - Vhat: (2048, 520) bf16 = per head [V_h (64 cols) | 1.0]; the ones column
    makes the PV matmul emit the softmax denominator as row 64
  - scores: row-tiled concurrent matmul pairs (two heads in PE row groups
    0-63/64-127, contract dim Dh=64) -> one [128, 1024] PSUM set so exp
    runs at FD=1024; P written in bf16 straight from the activation
  - normalize: reciprocal of row 64 of the SBUF PV copy, partition-broadcast
    via DRAM bounce, multiply rows 0..63 -> attnT (512, 2048) bf16
  - out-projection: attnT chunks stationary x woT moving -> out (2048, 1024)
    fp32 partial, bias added on head-group-0 cores only.
"""

import numpy as np
import ml_dtypes

B = 4
S = 2048          # SQ == SKV
D = 1024
H_PER_CORE = 8
DH = 64
DC = H_PER_CORE * DH            # 512 head-concat dims per core
DHP = DH + 1                    # V-hat column block per head (64 + ones col)
N_CORES = 8

_CACHE = {}


def _build_program(repeat=1):
    import concourse.bass as bass
    import concourse.tile as tile
    from concourse import bacc, mybir

    f32 = mybir.dt.float32
    bf16 = mybir.dt.bfloat16
    nc = bacc.Bacc("TRN2", target_bir_lowering=False, debug=False,
                   enable_asserts=False, num_devices=N_CORES)

    qT = nc.dram_tensor("qT", [D, S], bf16, kind="ExternalInput").ap()
    kvT = nc.dram_tensor("kvT", [D, S], bf16, kind="ExternalInput").ap()
    wqT = nc.dram_tensor("wqT", [D, DC], bf16, kind="ExternalInput").ap()
    wkT = nc.dram_tensor("wkT", [D, DC], bf16, kind="ExternalInput").ap()
    wvh = nc.dram_tensor("wvh", [D, H_PER_CORE * DHP], bf16, kind="ExternalInput").ap()
    bq = nc.dram_tensor("bq", [DC], f32, kind="ExternalInput").ap()
    bk = nc.dram_tensor("bk", [DC], f32, kind="ExternalInput").ap()
    bvh = nc.dram_tensor("bvh", [H_PER_CORE * DHP], f32, kind="ExternalInput").ap()
    woT = nc.dram_tensor("woT", [DC, D], bf16, kind="ExternalInput").ap()
    bo = nc.dram_tensor("bo", [D], f32, kind="ExternalInput").ap()
    out = nc.dram_tensor("out", [S, D], f32, kind="ExternalOutput").ap()

    VW = H_PER_CORE * DHP       # 520
    KC = D // 128               # 8 contraction chunks for projections
    NM = DC // 128              # 4 partition chunks of QT/KT (head pairs)
    JW = 512                    # q-chunk width
    NJ = S // JW                # 4 q-chunks
    NSB = S // 128              # 16 s-blocks

    with tile.TileContext(nc) as tc:
      def _emit():
        with tc.tile_pool(name="persist", bufs=1) as persist, \
             tc.tile_pool(name="wqo", bufs=1) as wqo, \
             tc.tile_pool(name="xq", bufs=2) as xq, \
             tc.tile_pool(name="ppq", bufs=2, space="PSUM") as ppq, \
             tc.tile_pool(name="otp", bufs=3) as otp:
            qt_t = [persist.tile([128, S], bf16, tag=f"qt{m}", name=f"qt{m}") for m in range(NM)]
            kt_t = [persist.tile([128, S], bf16, tag=f"kt{m}", name=f"kt{m}") for m in range(NM)]
            vh_t = [persist.tile([128, VW], bf16, tag=f"vh{sb}", name=f"vh{sb}") for sb in range(NSB)]
            at_t = [persist.tile([128, S], bf16, tag=f"at{m}", name=f"at{m}") for m in range(NM)]

            # biases: bq/bk as (128, NM) per-partition scalars; bvh broadcast
            bq_t = persist.tile([128, NM], f32, tag="bq")
            bk_t = persist.tile([128, NM], f32, tag="bk")
            bvh_t = persist.tile([128, VW], f32, tag="bvh")
            bo_t = persist.tile([128, D], f32, tag="bo")

            def col_ap(vec, n):  # (n*128,) dram vector -> (128, n) column tile ap
                return bass.AP(tensor=vec.tensor, offset=vec.offset,
                               ap=[[1, 128], [128, n]])

            def bcast_ap(vec, p, w):  # (w,) dram vector -> (p, w) broadcast
                return bass.AP(tensor=vec.tensor, offset=vec.offset,
                               ap=[[0, p], [1, w]])

            nc.sync.dma_start(out=bq_t, in_=col_ap(bq, NM))
            nc.sync.dma_start(out=bk_t, in_=col_ap(bk, NM))
            nc.sync.dma_start(out=bvh_t, in_=bcast_ap(bvh, 128, VW))
            nc.sync.dma_start(out=bo_t, in_=bcast_ap(bo, 128, D))

            # weights that live through the whole kernel: Wq + Wo
            wq_t = [wqo.tile([128, DC], bf16, tag=f"wq{k}", name=f"wq{k}") for k in range(KC)]
            wo_t = [wqo.tile([128, D], bf16, tag=f"wo{k}", name=f"wo{k}") for k in range(NM)]
            for k in range(KC):
                nc.sync.dma_start(out=wq_t[k], in_=wqT[k * 128:(k + 1) * 128, :])
            for k in range(NM):
                nc.sync.dma_start(out=wo_t[k], in_=woT[k * 128:(k + 1) * 128, :])

            def q_dma(jc):
                """Start the qT column loads for q-chunk jc."""
                jsl = slice(jc * JW, (jc + 1) * JW)
                q_c = [xq.tile([128, JW], bf16, tag=f"q{k}", name=f"q{k}")
                       for k in range(KC)]
                for k in range(KC):
                    nc.sync.dma_start(out=q_c[k], in_=qT[k * 128:(k + 1) * 128, jsl])
                return q_c

            def q_proj_group(jc, m, q_c):
                """One m-chunk of the Q projection for q-chunk jc (~854ns PE)."""
                def emit():
                    jsl = slice(jc * JW, (jc + 1) * JW)
                    msl = slice(m * 128, (m + 1) * 128)
                    ps = ppq.tile([128, JW], f32, tag="proj")
                    for k in range(KC):
                        nc.tensor.matmul(ps, wq_t[k][:, msl], q_c[k],
                                         start=(k == 0), stop=(k == KC - 1))
                    nc.vector.tensor_scalar_add(qt_t[m][:, jsl], ps, bq_t[:, m:m + 1])
                return emit

            def out_proj_group(qm, n):
                """One (q-row-block, out-half) of the out-projection (~854ns PE)."""
                def emit():
                    qsl = slice(qm * 128, (qm + 1) * 128)
                    nsl = slice(n * 512, (n + 1) * 512)
                    po = ppq.tile([128, 512], f32, tag="proj")
                    for k in range(NM):
                        nc.tensor.matmul(po, at_t[k][:, qsl], wo_t[k][:, nsl],
                                         start=(k == 0), stop=(k == NM - 1))
                    o_t = otp.tile([128, 512], f32, tag="o")
                    nc.vector.tensor_add(o_t, po, bo_t[:, nsl])
                    nc.sync.dma_start(out=out[qsl, nsl], in_=o_t)
                return emit

            # ---- phase 1: K/V projections over 4 s-quarters, then Q(jc=0) --
            SQW = 512
            with tc.tile_pool(name="wkv", bufs=1) as wkv, \
                 tc.tile_pool(name="xkv", bufs=1) as xkv, \
                 tc.tile_pool(name="ppv", bufs=2, space="PSUM") as ppv:
                wk_t = [wkv.tile([128, DC], bf16, tag=f"wk{k}", name=f"wk{k}") for k in range(KC)]
                wv_t = [wkv.tile([128, VW], bf16, tag=f"wv{k}", name=f"wv{k}") for k in range(KC)]
                for k in range(KC):
                    nc.sync.dma_start(out=wk_t[k], in_=wkT[k * 128:(k + 1) * 128, :])
                    nc.sync.dma_start(out=wv_t[k], in_=wvh[k * 128:(k + 1) * 128, :])

                for sq in range(S // SQW):
                    ssl = slice(sq * SQW, (sq + 1) * SQW)
                    kv_c = [xkv.tile([128, SQW], bf16, tag=f"kv{k}", name=f"kv{k}")
                            for k in range(KC)]
                    for k in range(KC):
                        nc.sync.dma_start(out=kv_c[k], in_=kvT[k * 128:(k + 1) * 128, ssl])

                    for m in range(NM):
                        msl = slice(m * 128, (m + 1) * 128)
                        ps = ppq.tile([128, SQW], f32, tag="proj")
                        for k in range(KC):
                            nc.tensor.matmul(ps, wk_t[k][:, msl], kv_c[k],
                                             start=(k == 0), stop=(k == KC - 1))
                        nc.vector.tensor_scalar_add(kt_t[m][:, ssl], ps, bk_t[:, m:m + 1])
                    for sm in range(SQW // 128):
                        sb = sq * (SQW // 128) + sm
                        smsl = slice(sm * 128, (sm + 1) * 128)
                        psv = ppv.tile([128, 1024], f32, tag="vproj")
                        for k in range(KC):
                            nc.tensor.matmul(psv[:, 0:512], kv_c[k][:, smsl], wv_t[k][:, 0:512],
                                             start=(k == 0), stop=(k == KC - 1))
                            nc.tensor.matmul(psv[:, 512:VW], kv_c[k][:, smsl], wv_t[k][:, 512:VW],
                                             start=(k == 0), stop=(k == KC - 1))
                        nc.vector.tensor_add(vh_t[sb], psv[:, 0:VW], bvh_t)

            q_c0 = q_dma(0)
            for m in range(NM):
                q_proj_group(0, m, q_c0)()

            # ---- phase 2: attention with interleaved Q/out projections ----
            # Projection work is sliced into ~854ns PE groups and dropped
            # into the score-step stream (one group per ~5 s-blocks), filling
            # the PE slack under the ACT-bound exp cadence.
            with tc.tile_pool(name="sps", bufs=2, space="PSUM") as sps, \
                 tc.tile_pool(name="pvs", bufs=1, space="PSUM") as pvs, \
                 tc.tile_pool(name="pt", bufs=3) as ptp, \
                 tc.tile_pool(name="pvc", bufs=2) as pvcp, \
                 tc.tile_pool(name="nrm", bufs=3) as nrm, \
                 tc.tile_pool(name="dscr", bufs=3, space="DRAM") as dscr:
                for jc in range(NJ):
                    jsl = slice(jc * JW, (jc + 1) * JW)
                    pending = []
                    if jc > 0:
                        pending += [out_proj_group(qm, n)
                                    for qm in range((jc - 1) * (JW // 128),
                                                    jc * (JW // 128))
                                    for n in range(D // 512)]
                    if jc + 1 < NJ:
                        q_c = q_dma(jc + 1)
                        qg = [q_proj_group(jc + 1, m, q_c) for m in range(NM)]
                        # alternate q-proj groups among the out-proj groups
                        mixed = []
                        while pending or qg:
                            if pending:
                                mixed.append(pending.pop(0))
                            if qg:
                                mixed.append(qg.pop(0))
                        pending = mixed
                    for hp in range(NM):
                        v0 = slice(2 * hp * DHP, 2 * hp * DHP + DHP)
                        v1 = slice((2 * hp + 1) * DHP, (2 * hp + 1) * DHP + DHP)
                        pv0 = pvs.tile([DHP, JW], f32, tag="pv0")
                        pv1 = pvs.tile([DHP, JW], f32, tag="pv1")
                        for sb in range(NSB):
                            sbsl = slice(sb * 128, (sb + 1) * 128)
                            sp = sps.tile([128, 2 * JW], f32, tag="sc")
                            # two heads concurrently in PE row groups 0/64
                            nc.tensor.matmul(sp[:, 0:JW],
                                             kt_t[hp][0:64, sbsl],
                                             qt_t[hp][0:64, jsl],
                                             start=True, stop=True)
                            nc.tensor.matmul(sp[:, JW:2 * JW],
                                             kt_t[hp][64:128, sbsl],
                                             qt_t[hp][64:128, jsl],
                                             start=True, stop=True)
                            p_t = ptp.tile([128, 2 * JW], bf16, tag="p")
                            nc.scalar.activation(p_t, sp, mybir.ActivationFunctionType.Exp,
                                                 scale=0.125)
                            nc.tensor.matmul(pv0, vh_t[sb][:, v0], p_t[:, 0:JW],
                                             start=(sb == 0), stop=(sb == NSB - 1))
                            nc.tensor.matmul(pv1, vh_t[sb][:, v1], p_t[:, JW:2 * JW],
                                             start=(sb == 0), stop=(sb == NSB - 1))
                            if sb in (4, 9, 14) and pending:
                                pending.pop(0)()
                        # free the PSUM accumulators fast: raw copy to SBUF,
                        # normalize later from the copy
                        pvc0 = pvcp.tile([DHP, JW], f32, tag="pvc0")
                        pvc1 = pvcp.tile([DHP, JW], f32, tag="pvc1")
                        nc.vector.tensor_copy(pvc0, pv0)
                        nc.vector.tensor_copy(pvc1, pv1)
                        for h, pvc in ((0, pvc0), (1, pvc1)):
                            hsl = slice(h * 64, h * 64 + 64)
                            rec = nrm.tile([1, JW], f32, tag=f"rec{h}")
                            nc.vector.reciprocal(rec, pvc[64:65, :])
                            scr = dscr.tile([1, JW], f32, tag=f"scr{h}")
                            nc.sync.dma_start(out=scr, in_=rec)
                            recb = nrm.tile([64, JW], f32, tag=f"recb{h}")
                            sc = scr[0, :]
                            nc.sync.dma_start(
                                out=recb,
                                in_=bass.AP(tensor=sc.tensor, offset=sc.offset,
                                            ap=[[0, 64]] + sc.ap))
                            # on GPSIMD: keeps the DVE queue free of DMA waits
                            # so the PV-freeing copies issue promptly
                            nc.gpsimd.tensor_mul(at_t[hp][hsl, jsl], pvc[0:64, :], recb)
                    while pending:
                        pending.pop(0)()
                for qm in range((NJ - 1) * (JW // 128), NJ * (JW // 128)):
                    for n in range(D // 512):
                        out_proj_group(qm, n)()

      if repeat > 1:
          with tc.For_i(0, repeat, 1):
              _emit()
      else:
          _emit()

    nc.compile()
    return nc


def _get_runner(repeat=1):
    """Build the program once and return a cached jitted SPMD runner."""
    key = ("runner", repeat)
    if key in _CACHE:
        return _CACHE[key]

    import jax
    import jax.numpy as jnp
    from jax.sharding import Mesh, PartitionSpec
    from jax.experimental.shard_map import shard_map
    from concourse import mybir
    from concourse.bass2jax import (_bass_exec_p, install_neuronx_cc_hook,
                                    partition_id_tensor)

    nc = _build_program(repeat)
    install_neuronx_cc_hook()

    partition_name = nc.partition_id_tensor.name if nc.partition_id_tensor else None
    in_names, out_names, out_avals, zero_shapes = [], [], [], []
    for alloc in nc.m.functions[0].allocations:
        if not isinstance(alloc, mybir.MemoryLocationSet):
            continue
        name = alloc.memorylocations[0].name
        if alloc.kind == "ExternalInput":
            if name != partition_name:
                in_names.append(name)
        elif alloc.kind == "ExternalOutput":
            out_names.append(name)
            shape = tuple(alloc.tensor_shape)
            dtype = mybir.dt.np(alloc.dtype)
            out_avals.append(jax.core.ShapedArray(shape, dtype))
            zero_shapes.append((shape, dtype))
    n_params = len(in_names)
    n_outs = len(out_avals)
    all_in_names = list(in_names) + list(out_names)
    if partition_name is not None:
        all_in_names.append(partition_name)
    donate = tuple(range(n_params, n_params + n_outs))

    def _body(*args):
        operands = list(args)
        if partition_name is not None:
            operands.append(partition_id_tensor())
        outs = _bass_exec_p.bind(
            *operands,
            out_avals=tuple(out_avals),
            in_names=tuple(all_in_names),
            out_names=tuple(out_names),
            lowering_input_output_aliases=(),
            sim_require_finite=True,
            sim_require_nnan=True,
            nc=nc,
        )
        return tuple(outs)

    devices = jax.devices()[:N_CORES]
    mesh = Mesh(np.asarray(devices), ("core",))
    in_specs = (PartitionSpec("core"),) * (n_params + n_outs)
    out_specs = (PartitionSpec("core"),) * n_outs
    sharded = jax.jit(
        shard_map(_body, mesh=mesh, in_specs=in_specs, out_specs=out_specs,
                  check_rep=False),
        donate_argnums=donate, keep_unused=True)

    def run(in_maps):
        concat_in = [np.concatenate([np.asarray(m[name]) for m in in_maps], axis=0)
                     for name in in_names]
        concat_zeros = [np.zeros((N_CORES * s[0], *s[1:]), d) for s, d in zero_shapes]
        out_arrs = sharded(*concat_in, *concat_zeros)
        out_arrs = [np.asarray(a) for a in jax.block_until_ready(out_arrs)]
        return [
            {name: out_arrs[i].reshape(N_CORES, *out_avals[i].shape)[c]
             for i, name in enumerate(out_names)}
            for c in range(N_CORES)
        ]

    _CACHE[("internals", repeat)] = {
        "sharded": sharded, "mesh": mesh, "in_names": in_names,
        "out_names": out_names, "zero_shapes": zero_shapes, "nc": nc,
    }
    _CACHE[key] = run
    return run


def _prep_in_maps(query, key_value, Wq, bq, Wk, bk, Wv, bv, Wo, bo):
    f = np.float32
    bf = ml_dtypes.bfloat16
    in_maps = []
    for c in range(N_CORES):
        b, hg = c // 2, c % 2
        sl = slice(hg * DC, (hg + 1) * DC)
        wv_s = np.asarray(Wv, f)[sl, :].T.reshape(D, H_PER_CORE, DH)
        wvh = np.concatenate([wv_s, np.zeros((D, H_PER_CORE, 1), f)], axis=2)
        bv_s = np.asarray(bv, f)[sl].reshape(H_PER_CORE, DH)
        bvh = np.concatenate([bv_s, np.ones((H_PER_CORE, 1), f)], axis=1)
        in_maps.append({
            "qT": np.ascontiguousarray(np.asarray(query, f)[b].T).astype(bf),
            "kvT": np.ascontiguousarray(np.asarray(key_value, f)[b].T).astype(bf),
            "wqT": np.ascontiguousarray(np.asarray(Wq, f)[sl, :].T).astype(bf),
            "wkT": np.ascontiguousarray(np.asarray(Wk, f)[sl, :].T).astype(bf),
            "wvh": np.ascontiguousarray(wvh.reshape(D, H_PER_CORE * DHP)).astype(bf),
            "bq": np.ascontiguousarray(np.asarray(bq, f)[sl]),
            "bk": np.ascontiguousarray(np.asarray(bk, f)[sl]),
            "bvh": np.ascontiguousarray(bvh.reshape(H_PER_CORE * DHP)),
            "woT": np.ascontiguousarray(np.asarray(Wo, f)[:, sl].T).astype(bf),
            "bo": (np.asarray(bo, f) if hg == 0 else np.zeros(D, f)),
        })
    return in_maps


def kernel(query, key_value, Wq, bq, Wk, bk, Wv, bv, Wo, bo):
    run = _get_runner()
    in_maps = _prep_in_maps(query, key_value, Wq, bq, Wk, bk, Wv, bv, Wo, bo)
    results = run(in_maps)
    out = np.empty((B, S, D), np.float32)
    for b in range(B):
        out[b] = results[2 * b]["out"] + results[2 * b + 1]["out"]
    return out
